# revision 3
# baseline (speedup 1.0000x reference)
"""v3 Trainium2 Bass kernel for nn_AnomalyDetector (8-layer SimpleRNN AE).

Wavefront pipeline (layer l computes step t at wavefront tau = t + l), with
all per-wavefront data movement on cheap-latency paths:

  Packs (5 matmuls / wavefront, f32r, moving N=2048 cols, all base-0):
    A  = {L0,L1}    SA[0:128] = [h1(32); h0(64); x-strip(32)] -> [h1';h0']
    M  = {L2..L5}   SM[0:128] = [h5;h4;h3;h2;pad;h1dup]       -> [h5'..h2']
    D  = {L6}       SD[0:96]  = [h6(64); h5dup(32)]           -> h6'
    E/F= {L7}       SE[0:79] (h7) + SD[0:64] (h6), psum accum -> h7'

  - x rows live in a 32-slot strip (SA partitions 96:128); pack A's
    stationary has 32 variants (Wx0 row at slot tau%32), so x needs only
    2 bulk refill DMAs (tau=32, 64) for the whole kernel — no per-wavefront
    x delivery.
  - Cross-pack dups (h1: A->M, h5: M->D) are per-half Pool-engine copies
    issued right after their producer evictions.  Pool copies signal with
    ~150ns semaphore latency (vs ~3us end-to-end for a DGE DMA, whose
    issue+transfer+sem-prop previously sat on the A-evict WAR cycle), so
    one wavefront of slack is enough.

Engine budget per wavefront (BL=2048, 1024-col evict chunks):
  PE  : 5 x 2048 cols                  = 10240 cyc @2.4GHz = 4.27us
  ACT : A + E + 0.25 D evictions       ~ 4.6us
  DVE : M + 0.75 D evictions           ~ 4.5us
  Pool: 4 half-copies (853ns each)     ~ 3.8us
  SP  : x refill every 32 wavefronts   ~ 0.1us
"""

import sys

import numpy as np

if "/opt/trn_rl_repo" not in sys.path:
    sys.path.insert(0, "/opt/trn_rl_repo")

B, T = 16384, 79
NCORES = 8
BL = B // NCORES

NSLOT = 32  # x-strip slots (SA partitions 96:128)

_NC_CACHE = {}


def _build_bass(reps=1):
    import concourse.bacc as bacc
    import concourse.mybir as mybir
    from concourse.tile import TileContext

    fp32 = mybir.dt.float32
    f32r = mybir.dt.float32r
    AF = mybir.ActivationFunctionType
    ALU = mybir.AluOpType

    nc = bacc.Bacc()

    xt_d = nc.declare_dram_parameter("xt", [T, BL], f32r, isOutput=False)
    wA_d = nc.declare_dram_parameter("wA", [128, NSLOT * 96], f32r, isOutput=False)
    wM_d = nc.declare_dram_parameter("wM", [128, 72], f32r, isOutput=False)
    wD_d = nc.declare_dram_parameter("wD", [96, 64], f32r, isOutput=False)
    wE_d = nc.declare_dram_parameter("wE", [79, 80], f32r, isOutput=False)
    wF_d = nc.declare_dram_parameter("wF", [64, 80], f32r, isOutput=False)
    bA_d = nc.declare_dram_parameter("bA", [96, 1], fp32, isOutput=False)
    bM_d = nc.declare_dram_parameter("bM", [72, 1], fp32, isOutput=False)
    bD_d = nc.declare_dram_parameter("bD", [64, 1], fp32, isOutput=False)
    bE_d = nc.declare_dram_parameter("bE", [79, 1], fp32, isOutput=False)
    zz_d = nc.declare_dram_parameter("zz", [128, BL], f32r, isOutput=False)
    out_d = nc.declare_dram_parameter("out", [BL, T], fp32, isOutput=True)

    with TileContext(nc) as tc:
        with (
            tc.tile_pool(name="const", bufs=1) as cpool,
            tc.tile_pool(name="state", bufs=1) as spool,
            tc.tile_pool(name="ps", bufs=_NC_CACHE.get("psum_bufs", 4), space="PSUM") as pspool,
            tc.tile_pool(name="ostage", bufs=4) as opool,
        ):
            wA = cpool.tile([128, NSLOT * 96], f32r, name="wA_sb")
            wM = cpool.tile([128, 72], f32r, name="wM_sb")
            wD = cpool.tile([96, 64], f32r, name="wD_sb")
            wE = cpool.tile([79, 80], f32r, name="wE_sb")
            wF = cpool.tile([64, 80], f32r, name="wF_sb")
            bA = cpool.tile([96, 1], fp32, name="bA_sb")
            bM = cpool.tile([72, 1], fp32, name="bM_sb")
            bD = cpool.tile([64, 1], fp32, name="bD_sb")
            bE = cpool.tile([79, 1], fp32, name="bE_sb")
            for sb, dr in ((wA, wA_d), (wM, wM_d), (wD, wD_d), (wE, wE_d),
                           (wF, wF_d), (bA, bA_d), (bM, bM_d), (bD, bD_d),
                           (bE, bE_d)):
                nc.sync.dma_start(out=sb[:, :], in_=dr[:, :])

            # persistent state tiles (transposed: [units, batch])
            SA = spool.tile([128, BL], f32r, name="SA")
            SM = spool.tile([128, BL], f32r, name="SM")
            SD = spool.tile([96, BL], f32r, name="SD")
            SE = spool.tile([79, BL], f32r, name="SE")
            nc.sync.dma_start(out=SA[0:96, :], in_=zz_d[0:96, :])
            nc.sync.dma_start(out=SM[0:128, :], in_=zz_d[0:128, :])
            nc.sync.dma_start(out=SD[0:96, :], in_=zz_d[0:96, :])
            nc.sync.dma_start(out=SE[0:79, :], in_=zz_d[0:79, :])
            nc.sync.dma_start(out=SA[96:128, :], in_=xt_d[0:32, :])

            PW = _NC_CACHE.get("psum_w", 1024)
            HALF = PW
            NH = BL // HALF
            NQ = HALF // 512

            def mm(ps_ap, w_ap, mv_ap, start=True, stop=True):
                nc.tensor.matmul(ps_ap, w_ap, mv_ap, start=start, stop=stop)

            for rep in range(reps):
              for tau in range(0, 85):
                  emitA = tau <= 79
                  emitM = 2 <= tau <= 83
                  emitD = 6 <= tau
                  emitE = 7 <= tau

                  # x-strip refill (SP DMA): 2 per kernel, WAR-serialized
                  # against the previous epoch's last A matmul only
                  if tau in (32, 64) and not _NC_CACHE.get("no_xdma"):
                      hi = min(tau + 32, T) - tau
                      nc.sync.dma_start(out=SA[96:96 + hi, :],
                                        in_=xt_d[tau:tau + hi, :])

                  wAv = wA[:, (tau % NSLOT) * 96:(tau % NSLOT + 1) * 96]

                  psA, psM_, psD, psE = [None]*NH, [None]*NH, [None]*NH, [None]*NH

                  def emit_pack(p, h):
                      c0 = h * HALF
                      if p == "A" and emitA:
                          psA[h] = pspool.tile([96, HALF], fp32, tag="ps", name="psA")
                          for q in range(NQ):
                              s = c0 + q * 512
                              mm(psA[h][:, q * 512:(q + 1) * 512], wAv,
                                 SA[0:128, s:s + 512])
                      if p == "M" and emitM:
                          psM_[h] = pspool.tile([72, HALF], fp32, tag="ps", name="psM")
                          for q in range(NQ):
                              s = c0 + q * 512
                              mm(psM_[h][:, q * 512:(q + 1) * 512], wM[:, :],
                                 SM[0:128, s:s + 512])
                      if p == "D" and emitD:
                          psD[h] = pspool.tile([64, HALF], fp32, tag="ps", name="psD")
                          for q in range(NQ):
                              s = c0 + q * 512
                              mm(psD[h][:, q * 512:(q + 1) * 512], wD[:, :],
                                 SD[0:96, s:s + 512])
                      if p == "E" and emitE:
                          psE[h] = pspool.tile([79, HALF], fp32, tag="ps", name="psE")
                          for q in range(NQ):
                              s = c0 + q * 512
                              mm(psE[h][:, q * 512:(q + 1) * 512], wE[:, 0:79],
                                 SE[0:79, s:s + 512], start=True, stop=False)
                              mm(psE[h][:, q * 512:(q + 1) * 512], wF[:, 0:79],
                                 SD[0:64, s:s + 512], start=False, stop=True)

                  order = _NC_CACHE.get("order", [(p, h) for h in range(NH)
                                                  for p in "EAMD"])
                  for p, h in order:
                      emit_pack(p, h)

                  # Evictions.  ACT: A (first, so dups start early), E,
                  # D-tail.  DVE: M (first), D-head.  Dups: per-half Pool
                  # copies right after their producer evictions.
                  for h in range(NH):
                      cols = slice(h * HALF, (h + 1) * HALF)
                      if emitA:
                          nc.scalar.activation(
                              SA[0:96, cols], psA[h][0:96, :], AF.Relu,
                              bias=bA[0:96, 0:1],
                          )
                          if not _NC_CACHE.get("no_dup"):
                              nc.gpsimd.tensor_copy(SM[96:128, cols],
                                                    SA[0:32, cols])
                  for h in range(NH):
                      cols = slice(h * HALF, (h + 1) * HALF)
                      if emitM:
                          nc.vector.tensor_scalar(
                              SM[0:72, cols], psM_[h][0:72, :],
                              bM[0:72, 0:1], 0.0, ALU.add, ALU.max,
                          )
                          if not _NC_CACHE.get("no_dup"):
                              nc.gpsimd.tensor_copy(SD[64:96, cols],
                                                    SM[0:32, cols])
                  for h in range(NH):
                      c0 = h * HALF
                      if emitD:
                          if h < NH - 1:
                              nc.vector.tensor_scalar(
                                  SD[0:64, c0:c0 + HALF], psD[h][0:64, :],
                                  bD[0:64, 0:1], 0.0, ALU.add, ALU.max,
                              )
                          else:
                              hw_ = HALF // 2
                              nc.vector.tensor_scalar(
                                  SD[0:64, c0:c0 + hw_], psD[h][0:64, 0:hw_],
                                  bD[0:64, 0:1], 0.0, ALU.add, ALU.max,
                              )
                              nc.scalar.activation(
                                  SD[0:64, c0 + hw_:c0 + HALF],
                                  psD[h][0:64, hw_:HALF],
                                  AF.Relu, bias=bD[0:64, 0:1],
                              )
                  for h in range(NH):
                      cols = slice(h * HALF, (h + 1) * HALF)
                      if emitE:
                          nc.scalar.activation(
                              SE[0:79, cols], psE[h][0:79, :], AF.Sigmoid,
                              bias=bE[0:79, 0:1],
                          )

            # final step: t=78 of L7 in batch-partition layout -> out [BL, 79]
            for c in range(16):
                csl = slice(c * 128, (c + 1) * 128)
                psO = pspool.tile([128, 80], fp32, tag="ps", name="psO")
                mm(psO[:, :], SE[0:79, csl], wE[:, :], start=True, stop=False)
                mm(psO[:, :], SD[0:64, csl], wF[:, :], start=False, stop=True)
                ob = opool.tile([128, 80], fp32, tag="ob", name="ob")
                nc.scalar.activation(ob[:, :], psO[:, :], AF.Sigmoid)
                nc.sync.dma_start(out=out_d[csl, :], in_=ob[:, 0:79])

    nc.compile()
    return nc


def _get_nc(reps=1):
    key = ("nc", reps)
    if key not in _NC_CACHE:
        _NC_CACHE[key] = _build_bass(reps)
    return _NC_CACHE[key]


def _pack_inputs(inputs):
    g = lambda k: np.ascontiguousarray(np.asarray(inputs[k], dtype=np.float32))
    Wx = [g(f"Wx{i}") for i in range(8)]
    Wh = [g(f"Wh{i}") for i in range(8)]
    b = [g(f"b{i}") for i in range(8)]

    # pack A [128, 96] x 32 variants; SA rows h1@0:32 h0@32:96 x@96+i;
    # psA cols h1'@0:32 h0'@32:96
    wA = np.zeros((128, NSLOT * 96), np.float32)
    for i in range(NSLOT):
        blk = wA[:, i * 96:(i + 1) * 96]
        blk[0:32, 0:32] = Wh[1]
        blk[32:96, 0:32] = Wx[1]
        blk[32:96, 32:96] = Wh[0]
        blk[96 + i, 32:96] = Wx[0][0]

    # pack M: SM rows h5@0:32 h4@32:48 h3@48:56 h2@56:72 pad@72:96
    # h1dup@96:128; psM cols h5'@0:32 h4'@32:48 h3'@48:56 h2'@56:72
    wM = np.zeros((128, 72), np.float32)
    wM[0:32, 0:32] = Wh[5]
    wM[32:48, 0:32] = Wx[5]
    wM[32:48, 32:48] = Wh[4]
    wM[48:56, 32:48] = Wx[4]
    wM[48:56, 48:56] = Wh[3]
    wM[56:72, 48:56] = Wx[3]
    wM[56:72, 56:72] = Wh[2]
    wM[96:128, 56:72] = Wx[2]

    wD = np.zeros((96, 64), np.float32)
    wD[0:64, :] = Wh[6]
    wD[64:96, :] = Wx[6]

    wE = np.zeros((79, 80), np.float32)
    wE[:, 0:79] = Wh[7]
    wF = np.zeros((64, 80), np.float32)
    wF[:, 0:79] = Wx[7]

    bA = np.concatenate([b[1], b[0]]).reshape(96, 1).astype(np.float32)
    bM = np.concatenate([b[5], b[4], b[3], b[2]]).reshape(72, 1).astype(np.float32)
    bD = b[6].reshape(64, 1).astype(np.float32)
    bE = b[7].reshape(79, 1).astype(np.float32)

    zz = np.zeros((128, BL), np.float32)
    common = dict(wA=wA, wM=wM, wD=wD, wE=wE, wF=wF, bA=bA, bM=bM, bD=bD,
                  bE=bE, zz=zz)

    x = np.asarray(inputs["x"], dtype=np.float32)
    in_maps = []
    for c in range(NCORES):
        xs = x[c * BL:(c + 1) * BL]
        m = dict(common)
        m["xt"] = np.ascontiguousarray(xs.T).astype(np.float32)
        in_maps.append(m)
    return in_maps


def run(inputs, trace=False, **kw):
    from concourse.bass_utils import run_bass_kernel_spmd

    nc = _get_nc()
    in_maps = _pack_inputs(inputs)
    res = run_bass_kernel_spmd(nc, in_maps, core_ids=list(range(NCORES)),
                               trace=trace, **kw)
    out = np.concatenate([res.results[c]["out"] for c in range(NCORES)], axis=0)
    return out.astype(np.float32), res


def kernel(**inputs) -> np.ndarray:
    out, _ = run(inputs, trace=False)
    return out


# revision 5
# speedup vs baseline: 1.7024x; 1.7024x over previous
"""v3 Trainium2 Bass kernel for nn_AnomalyDetector (8-layer SimpleRNN AE).

Wavefront pipeline (layer l computes step t at wavefront tau = t + l), with
all per-wavefront data movement on cheap-latency paths:

  Packs (5 matmuls / wavefront, f32r, moving N=2048 cols, all base-0):
    A  = {L0,L1}    SA[0:128] = [h1(32); h0(64); x-strip(32)] -> [h1';h0']
    M  = {L2..L5}   SM[0:128] = [h5;h4;h3;h2;pad;h1dup]       -> [h5'..h2']
    D  = {L6}       SD[0:96]  = [h6(64); h5dup(32)]           -> h6'
    E/F= {L7}       SE[0:79] (h7) + SD[0:64] (h6), psum accum -> h7'

  - x rows live in a 32-slot strip (SA partitions 96:128); pack A's
    stationary has 32 variants (Wx0 row at slot tau%32), so x needs only
    2 bulk refill DMAs (tau=32, 64) for the whole kernel — no per-wavefront
    x delivery.
  - Cross-pack dups (h1: A->M, h5: M->D) are per-half Pool-engine copies
    issued right after their producer evictions.  Pool copies signal with
    ~150ns semaphore latency (vs ~3us end-to-end for a DGE DMA, whose
    issue+transfer+sem-prop previously sat on the A-evict WAR cycle), so
    one wavefront of slack is enough.

Engine budget per wavefront (BL=2048, 1024-col evict chunks):
  PE  : 5 x 2048 cols                  = 10240 cyc @2.4GHz = 4.27us
  ACT : A + E + 0.25 D evictions       ~ 4.6us
  DVE : M + 0.75 D evictions           ~ 4.5us
  Pool: 4 half-copies (853ns each)     ~ 3.8us
  SP  : x refill every 32 wavefronts   ~ 0.1us
"""

import sys

import numpy as np

if "/opt/trn_rl_repo" not in sys.path:
    sys.path.insert(0, "/opt/trn_rl_repo")

B, T = 16384, 79
NCORES = 8
BL = B // NCORES

NSLOT = 32  # x-strip slots (SA partitions 96:128)

_NC_CACHE = {}


def _build_bass(reps=1):
    import concourse.bacc as bacc
    import concourse.mybir as mybir
    from concourse.tile import TileContext

    fp32 = mybir.dt.float32
    f32r = (mybir.dt.float32r if _NC_CACHE.get("cdt") == "f32r"
            else mybir.dt.bfloat16)
    AF = mybir.ActivationFunctionType
    ALU = mybir.AluOpType

    nc = bacc.Bacc()

    xt_d = nc.declare_dram_parameter("xt", [T, BL], f32r, isOutput=False)
    wA_d = nc.declare_dram_parameter("wA", [128, NSLOT * 96], f32r, isOutput=False)
    wM_d = nc.declare_dram_parameter("wM", [128, 72], f32r, isOutput=False)
    wD_d = nc.declare_dram_parameter("wD", [96, 64], f32r, isOutput=False)
    wE_d = nc.declare_dram_parameter("wE", [79, 80], f32r, isOutput=False)
    wF_d = nc.declare_dram_parameter("wF", [64, 80], f32r, isOutput=False)
    bA_d = nc.declare_dram_parameter("bA", [96, 1], fp32, isOutput=False)
    bM_d = nc.declare_dram_parameter("bM", [72, 1], fp32, isOutput=False)
    bD_d = nc.declare_dram_parameter("bD", [64, 1], fp32, isOutput=False)
    bE_d = nc.declare_dram_parameter("bE", [79, 1], fp32, isOutput=False)
    zz_d = nc.declare_dram_parameter("zz", [128, BL], f32r, isOutput=False)
    out_d = nc.declare_dram_parameter("out", [BL, T], fp32, isOutput=True)

    with TileContext(nc) as tc:
        with (
            tc.tile_pool(name="const", bufs=1) as cpool,
            tc.tile_pool(name="state", bufs=1) as spool,
            tc.tile_pool(name="ps", bufs=_NC_CACHE.get("psum_bufs", 4), space="PSUM") as pspool,
            tc.tile_pool(name="ostage", bufs=4) as opool,
        ):
            wA = cpool.tile([128, NSLOT * 96], f32r, name="wA_sb")
            wM = cpool.tile([128, 72], f32r, name="wM_sb")
            wD = cpool.tile([96, 64], f32r, name="wD_sb")
            wE = cpool.tile([79, 80], f32r, name="wE_sb")
            wF = cpool.tile([64, 80], f32r, name="wF_sb")
            bA = cpool.tile([96, 1], fp32, name="bA_sb")
            bM = cpool.tile([72, 1], fp32, name="bM_sb")
            bD = cpool.tile([64, 1], fp32, name="bD_sb")
            bE = cpool.tile([79, 1], fp32, name="bE_sb")
            for sb, dr in ((wA, wA_d), (wM, wM_d), (wD, wD_d), (wE, wE_d),
                           (wF, wF_d), (bA, bA_d), (bM, bM_d), (bD, bD_d),
                           (bE, bE_d)):
                nc.sync.dma_start(out=sb[:, :], in_=dr[:, :])

            # persistent state tiles (transposed: [units, batch])
            SA = spool.tile([128, BL], f32r, name="SA")
            SM = spool.tile([128, BL], f32r, name="SM")
            SD = spool.tile([96, BL], f32r, name="SD")
            SE = spool.tile([79, BL], f32r, name="SE")
            nc.sync.dma_start(out=SA[0:96, :], in_=zz_d[0:96, :])
            nc.sync.dma_start(out=SM[0:128, :], in_=zz_d[0:128, :])
            nc.sync.dma_start(out=SD[0:96, :], in_=zz_d[0:96, :])
            nc.sync.dma_start(out=SE[0:79, :], in_=zz_d[0:79, :])
            nc.sync.dma_start(out=SA[96:128, :], in_=xt_d[0:32, :])

            PW = _NC_CACHE.get("psum_w", 1024)
            HALF = PW
            NH = BL // HALF
            NQ = HALF // 512

            def mm(ps_ap, w_ap, mv_ap, start=True, stop=True):
                nc.tensor.matmul(ps_ap, w_ap, mv_ap, start=start, stop=stop)

            for rep in range(reps):
              for tau in range(0, 85):
                  emitA = tau <= 79
                  emitM = 2 <= tau <= 83
                  emitD = 6 <= tau
                  emitE = 7 <= tau

                  # x-strip refill (SP DMA): 2 per kernel, WAR-serialized
                  # against the previous epoch's last A matmul only
                  if tau in (32, 64) and not _NC_CACHE.get("no_xdma"):
                      hi = min(tau + 32, T) - tau
                      nc.sync.dma_start(out=SA[96:96 + hi, :],
                                        in_=xt_d[tau:tau + hi, :])

                  wAv = wA[:, (tau % NSLOT) * 96:(tau % NSLOT + 1) * 96]

                  psA, psM_, psD, psE = [None]*NH, [None]*NH, [None]*NH, [None]*NH

                  def emit_pack(p, h):
                      c0 = h * HALF
                      if p == "A" and emitA:
                          psA[h] = pspool.tile([96, HALF], fp32, tag="ps", name="psA")
                          for q in range(NQ):
                              s = c0 + q * 512
                              mm(psA[h][:, q * 512:(q + 1) * 512], wAv,
                                 SA[0:128, s:s + 512])
                      if p == "M" and emitM:
                          psM_[h] = pspool.tile([72, HALF], fp32, tag="ps", name="psM")
                          for q in range(NQ):
                              s = c0 + q * 512
                              mm(psM_[h][:, q * 512:(q + 1) * 512], wM[:, :],
                                 SM[0:128, s:s + 512])
                      if p == "D" and emitD:
                          psD[h] = pspool.tile([64, HALF], fp32, tag="ps", name="psD")
                          for q in range(NQ):
                              s = c0 + q * 512
                              mm(psD[h][:, q * 512:(q + 1) * 512], wD[:, :],
                                 SD[0:96, s:s + 512])
                      if p == "E" and emitE:
                          psE[h] = pspool.tile([79, HALF], fp32, tag="ps", name="psE")
                          for q in range(NQ):
                              s = c0 + q * 512
                              mm(psE[h][:, q * 512:(q + 1) * 512], wE[:, 0:79],
                                 SE[0:79, s:s + 512], start=True, stop=False)
                              mm(psE[h][:, q * 512:(q + 1) * 512], wF[:, 0:79],
                                 SD[0:64, s:s + 512], start=False, stop=True)

                  order = _NC_CACHE.get("order", [(p, h) for h in range(NH)
                                                  for p in "EAMD"])
                  for p, h in order:
                      emit_pack(p, h)

                  # Evictions.  ACT: A (first, so dups start early), E,
                  # D-tail.  DVE: M (first), D-head.  Dups: per-half Pool
                  # copies right after their producer evictions.
                  for h in range(NH):
                      cols = slice(h * HALF, (h + 1) * HALF)
                      if emitA:
                          nc.scalar.activation(
                              SA[0:96, cols], psA[h][0:96, :], AF.Relu,
                              bias=bA[0:96, 0:1],
                          )
                  if emitA and not _NC_CACHE.get("no_dup"):
                      m1 = _NC_CACHE.get("dup1", "sp")
                      if m1 == "sp":
                          nc.sync.dma_start(out=SM[96:128, :], in_=SA[0:32, :])
                      elif m1 == "dvehalf":
                          nc.vector.tensor_copy(SM[96:128, 0:HALF], SA[0:32, 0:HALF])
                          nc.vector.tensor_copy(SM[96:128, HALF:BL], SA[0:32, HALF:BL])
                      else:
                          nc.vector.tensor_copy(SM[96:128, :], SA[0:32, :])
                  for h in range(NH):
                      cols = slice(h * HALF, (h + 1) * HALF)
                      if emitM:
                          nc.vector.tensor_scalar(
                              SM[0:72, cols], psM_[h][0:72, :],
                              bM[0:72, 0:1], 0.0, ALU.add, ALU.max,
                          )
                  if emitM and not _NC_CACHE.get("no_dup"):
                      m5 = _NC_CACHE.get("dup5", "sp")
                      if m5 == "sp":
                          nc.sync.dma_start(out=SD[64:96, :], in_=SM[0:32, :])
                      elif m5 == "dvehalf":
                          nc.vector.tensor_copy(SD[64:96, 0:HALF], SM[0:32, 0:HALF])
                          nc.vector.tensor_copy(SD[64:96, HALF:BL], SM[0:32, HALF:BL])
                      else:
                          nc.vector.tensor_copy(SD[64:96, :], SM[0:32, :])
                  for h in range(NH):
                      c0 = h * HALF
                      if emitD:
                          if h < NH - 1:
                              nc.vector.tensor_scalar(
                                  SD[0:64, c0:c0 + HALF], psD[h][0:64, :],
                                  bD[0:64, 0:1], 0.0, ALU.add, ALU.max,
                              )
                          else:
                              nc.scalar.activation(
                                  SD[0:64, c0:c0 + HALF], psD[h][0:64, :],
                                  AF.Relu, bias=bD[0:64, 0:1],
                              )
                  for h in range(NH):
                      cols = slice(h * HALF, (h + 1) * HALF)
                      if emitE:
                          nc.scalar.activation(
                              SE[0:79, cols], psE[h][0:79, :], AF.Sigmoid,
                              bias=bE[0:79, 0:1],
                          )

            # final step: t=78 of L7 in batch-partition layout -> out [BL, 79]
            for c in range(16):
                csl = slice(c * 128, (c + 1) * 128)
                psO = pspool.tile([128, 80], fp32, tag="ps", name="psO")
                mm(psO[:, :], SE[0:79, csl], wE[:, :], start=True, stop=False)
                mm(psO[:, :], SD[0:64, csl], wF[:, :], start=False, stop=True)
                ob = opool.tile([128, 80], fp32, tag="ob", name="ob")
                nc.scalar.activation(ob[:, :], psO[:, :], AF.Sigmoid)
                nc.sync.dma_start(out=out_d[csl, :], in_=ob[:, 0:79])

    nc.compile()
    return nc


def _get_nc(reps=1):
    key = ("nc", reps)
    if key not in _NC_CACHE:
        _NC_CACHE[key] = _build_bass(reps)
    return _NC_CACHE[key]


def _pack_inputs(inputs):
    g = lambda k: np.ascontiguousarray(np.asarray(inputs[k], dtype=np.float32))
    Wx = [g(f"Wx{i}") for i in range(8)]
    Wh = [g(f"Wh{i}") for i in range(8)]
    b = [g(f"b{i}") for i in range(8)]

    # pack A [128, 96] x 32 variants; SA rows h1@0:32 h0@32:96 x@96+i;
    # psA cols h1'@0:32 h0'@32:96
    wA = np.zeros((128, NSLOT * 96), np.float32)
    for i in range(NSLOT):
        blk = wA[:, i * 96:(i + 1) * 96]
        blk[0:32, 0:32] = Wh[1]
        blk[32:96, 0:32] = Wx[1]
        blk[32:96, 32:96] = Wh[0]
        blk[96 + i, 32:96] = Wx[0][0]

    # pack M: SM rows h5@0:32 h4@32:48 h3@48:56 h2@56:72 pad@72:96
    # h1dup@96:128; psM cols h5'@0:32 h4'@32:48 h3'@48:56 h2'@56:72
    wM = np.zeros((128, 72), np.float32)
    wM[0:32, 0:32] = Wh[5]
    wM[32:48, 0:32] = Wx[5]
    wM[32:48, 32:48] = Wh[4]
    wM[48:56, 32:48] = Wx[4]
    wM[48:56, 48:56] = Wh[3]
    wM[56:72, 48:56] = Wx[3]
    wM[56:72, 56:72] = Wh[2]
    wM[96:128, 56:72] = Wx[2]

    wD = np.zeros((96, 64), np.float32)
    wD[0:64, :] = Wh[6]
    wD[64:96, :] = Wx[6]

    wE = np.zeros((79, 80), np.float32)
    wE[:, 0:79] = Wh[7]
    wF = np.zeros((64, 80), np.float32)
    wF[:, 0:79] = Wx[7]

    bA = np.concatenate([b[1], b[0]]).reshape(96, 1).astype(np.float32)
    bM = np.concatenate([b[5], b[4], b[3], b[2]]).reshape(72, 1).astype(np.float32)
    bD = b[6].reshape(64, 1).astype(np.float32)
    bE = b[7].reshape(79, 1).astype(np.float32)

    cdt = np.float32
    if _NC_CACHE.get("cdt") != "f32r":
        import ml_dtypes
        cdt = ml_dtypes.bfloat16
        wA, wM, wD, wE, wF = (w.astype(cdt) for w in (wA, wM, wD, wE, wF))

    zz = np.zeros((128, BL), cdt)
    common = dict(wA=wA, wM=wM, wD=wD, wE=wE, wF=wF, bA=bA, bM=bM, bD=bD,
                  bE=bE, zz=zz)

    x = np.asarray(inputs["x"], dtype=np.float32)
    in_maps = []
    for c in range(NCORES):
        xs = x[c * BL:(c + 1) * BL]
        m = dict(common)
        m["xt"] = np.ascontiguousarray(xs.T).astype(cdt)
        in_maps.append(m)
    return in_maps


def run(inputs, trace=False, **kw):
    from concourse.bass_utils import run_bass_kernel_spmd

    nc = _get_nc()
    in_maps = _pack_inputs(inputs)
    res = run_bass_kernel_spmd(nc, in_maps, core_ids=list(range(NCORES)),
                               trace=trace, **kw)
    out = np.concatenate([res.results[c]["out"] for c in range(NCORES)], axis=0)
    return out.astype(np.float32), res


def kernel(**inputs) -> np.ndarray:
    out, _ = run(inputs, trace=False)
    return out


# revision 6
# speedup vs baseline: 2.6905x; 1.5804x over previous
"""v4 Trainium2 Bass kernel for nn_AnomalyDetector (8-layer SimpleRNN AE).

Wavefront pipeline (layer l computes step t at wavefront tau = t + l), with
all per-wavefront data movement on cheap-latency paths:

  Packs (5 matmuls / wavefront, f32r, moving N=2048 cols, all base-0):
    A  = {L0,L1}    SA[0:128] = [h1(32); h0(64); x-strip(32)] -> [h1';h0']
    M  = {L2..L5}   SM[0:128] = [h5;h4;h3;h2;pad;h1dup]       -> [h5'..h2']
    D  = {L6}       SD[0:96]  = [h6(64); h5dup(32)]           -> h6'
    E/F= {L7}       SE[0:79] (h7) + SD[0:64] (h6), psum accum -> h7'

  - x rows live in a 32-slot strip (SA partitions 96:128); pack A's
    stationary has 32 variants (Wx0 row at slot tau%32), so x needs only
    2 bulk refill DMAs (tau=32, 64) for the whole kernel — no per-wavefront
    x delivery.
  - Cross-pack dups (h1: A->M, h5: M->D) are full-width DVE tensor_copy
    ops (bf16 dense copies hit the 4x DVE mode: ~533ns each) issued right
    after their producer evictions.  Engine copies signal with ~150ns
    semaphore latency, so one wavefront of slack is enough; DGE DMA dups
    were measured both slower under fabric contention and latency-poisoned
    (issue+transfer+sem-prop ~3us on the A-evict WAR cycle).  Pool/GPSIMD
    copies are catastrophically slow on real hardware (~2us+ each).

Engine budget per wavefront (BL=2048, 1024-col evict chunks):
  PE  : 5 x 2048 cols                  = 10240 cyc @2.4GHz = 4.27us
  ACT : A + E + 0.25 D evictions       ~ 4.6us
  DVE : M + 0.75 D evictions + 2 dups  ~ 4.8us
  SP  : x refill every 32 wavefronts   ~ 0.1us
Measured (8-core SPMD, wall-slope over R-rep NEFFs): ~475-505us vs the
v1 baseline's ~751-787us in the same windows (0.63x).
"""

import sys

import numpy as np

if "/opt/trn_rl_repo" not in sys.path:
    sys.path.insert(0, "/opt/trn_rl_repo")

B, T = 16384, 79
NCORES = 8
BL = B // NCORES

NSLOT = 32  # x-strip slots (SA partitions 96:128)

_NC_CACHE = {}


def _build_bass(reps=1):
    import concourse.bacc as bacc
    import concourse.mybir as mybir
    from concourse.tile import TileContext

    fp32 = mybir.dt.float32
    f32r = (mybir.dt.float32r if _NC_CACHE.get("cdt") == "f32r"
            else mybir.dt.bfloat16)
    AF = mybir.ActivationFunctionType
    ALU = mybir.AluOpType

    nc = bacc.Bacc()

    xt_d = nc.declare_dram_parameter("xt", [T, BL], f32r, isOutput=False)
    wA_d = nc.declare_dram_parameter("wA", [128, NSLOT * 96], f32r, isOutput=False)
    wM_d = nc.declare_dram_parameter("wM", [128, 72], f32r, isOutput=False)
    wD_d = nc.declare_dram_parameter("wD", [96, 64], f32r, isOutput=False)
    wE_d = nc.declare_dram_parameter("wE", [79, 80], f32r, isOutput=False)
    wF_d = nc.declare_dram_parameter("wF", [64, 80], f32r, isOutput=False)
    bA_d = nc.declare_dram_parameter("bA", [96, 1], fp32, isOutput=False)
    bM_d = nc.declare_dram_parameter("bM", [72, 1], fp32, isOutput=False)
    bD_d = nc.declare_dram_parameter("bD", [64, 1], fp32, isOutput=False)
    bE_d = nc.declare_dram_parameter("bE", [79, 1], fp32, isOutput=False)
    zz_d = nc.declare_dram_parameter("zz", [128, BL], f32r, isOutput=False)
    out_d = nc.declare_dram_parameter("out", [BL, T], fp32, isOutput=True)

    with TileContext(nc) as tc:
        with (
            tc.tile_pool(name="const", bufs=1) as cpool,
            tc.tile_pool(name="state", bufs=1) as spool,
            tc.tile_pool(name="ps", bufs=_NC_CACHE.get("psum_bufs", 4), space="PSUM") as pspool,
            tc.tile_pool(name="ostage", bufs=4) as opool,
        ):
            wA = cpool.tile([128, NSLOT * 96], f32r, name="wA_sb")
            wM = cpool.tile([128, 72], f32r, name="wM_sb")
            wD = cpool.tile([96, 64], f32r, name="wD_sb")
            wE = cpool.tile([79, 80], f32r, name="wE_sb")
            wF = cpool.tile([64, 80], f32r, name="wF_sb")
            bA = cpool.tile([96, 1], fp32, name="bA_sb")
            bM = cpool.tile([72, 1], fp32, name="bM_sb")
            bD = cpool.tile([64, 1], fp32, name="bD_sb")
            bE = cpool.tile([79, 1], fp32, name="bE_sb")
            for sb, dr in ((wA, wA_d), (wM, wM_d), (wD, wD_d), (wE, wE_d),
                           (wF, wF_d), (bA, bA_d), (bM, bM_d), (bD, bD_d),
                           (bE, bE_d)):
                nc.sync.dma_start(out=sb[:, :], in_=dr[:, :])

            # persistent state tiles (transposed: [units, batch])
            SA = spool.tile([128, BL], f32r, name="SA")
            SM = spool.tile([128, BL], f32r, name="SM")
            SD = spool.tile([96, BL], f32r, name="SD")
            SE = spool.tile([79, BL], f32r, name="SE")
            nc.sync.dma_start(out=SA[0:96, :], in_=zz_d[0:96, :])
            nc.sync.dma_start(out=SM[0:128, :], in_=zz_d[0:128, :])
            nc.sync.dma_start(out=SD[0:96, :], in_=zz_d[0:96, :])
            nc.sync.dma_start(out=SE[0:79, :], in_=zz_d[0:79, :])
            nc.sync.dma_start(out=SA[96:128, :], in_=xt_d[0:32, :])

            PW = _NC_CACHE.get("psum_w", 1024)
            HALF = PW
            NH = BL // HALF
            NQ = HALF // 512

            def mm(ps_ap, w_ap, mv_ap, start=True, stop=True):
                nc.tensor.matmul(ps_ap, w_ap, mv_ap, start=start, stop=stop)

            for rep in range(reps):
              for tau in range(0, 85):
                  emitA = tau <= 79
                  emitM = 2 <= tau <= 83
                  emitD = 6 <= tau
                  emitE = 7 <= tau

                  # x-strip refill (SP DMA): 2 per kernel, WAR-serialized
                  # against the previous epoch's last A matmul only
                  if tau in (32, 64) and not _NC_CACHE.get("no_xdma"):
                      hi = min(tau + 32, T) - tau
                      nc.sync.dma_start(out=SA[96:96 + hi, :],
                                        in_=xt_d[tau:tau + hi, :])

                  wAv = wA[:, (tau % NSLOT) * 96:(tau % NSLOT + 1) * 96]

                  psA, psM_, psD, psE = [None]*NH, [None]*NH, [None]*NH, [None]*NH

                  def emit_pack(p, h):
                      c0 = h * HALF
                      if p == "A" and emitA:
                          psA[h] = pspool.tile([96, HALF], fp32, tag="ps", name="psA")
                          for q in range(NQ):
                              s = c0 + q * 512
                              mm(psA[h][:, q * 512:(q + 1) * 512], wAv,
                                 SA[0:128, s:s + 512])
                      if p == "M" and emitM:
                          psM_[h] = pspool.tile([72, HALF], fp32, tag="ps", name="psM")
                          for q in range(NQ):
                              s = c0 + q * 512
                              mm(psM_[h][:, q * 512:(q + 1) * 512], wM[:, :],
                                 SM[0:128, s:s + 512])
                      if p == "D" and emitD:
                          psD[h] = pspool.tile([64, HALF], fp32, tag="ps", name="psD")
                          for q in range(NQ):
                              s = c0 + q * 512
                              mm(psD[h][:, q * 512:(q + 1) * 512], wD[:, :],
                                 SD[0:96, s:s + 512])
                      if p == "E" and emitE:
                          psE[h] = pspool.tile([79, HALF], fp32, tag="ps", name="psE")
                          for q in range(NQ):
                              s = c0 + q * 512
                              mm(psE[h][:, q * 512:(q + 1) * 512], wE[:, 0:79],
                                 SE[0:79, s:s + 512], start=True, stop=False)
                              mm(psE[h][:, q * 512:(q + 1) * 512], wF[:, 0:79],
                                 SD[0:64, s:s + 512], start=False, stop=True)

                  order = _NC_CACHE.get("order", [(p, h) for h in range(NH)
                                                  for p in "EAMD"])
                  for p, h in order:
                      emit_pack(p, h)

                  # Evictions.  ACT: A (first, so dups start early), E,
                  # D-tail.  DVE: M (first), D-head.  Dups issued right
                  # after their producer evictions.
                  for h in range(NH):
                      cols = slice(h * HALF, (h + 1) * HALF)
                      if emitA:
                          nc.scalar.activation(
                              SA[0:96, cols], psA[h][0:96, :], AF.Relu,
                              bias=bA[0:96, 0:1],
                          )
                  if emitA and not _NC_CACHE.get("no_dup"):
                      m1 = _NC_CACHE.get("dup1", "dve")
                      if m1 == "sp":
                          nc.sync.dma_start(out=SM[96:128, :], in_=SA[0:32, :])
                      else:
                          nc.vector.tensor_copy(SM[96:128, :], SA[0:32, :])
                  for h in range(NH):
                      cols = slice(h * HALF, (h + 1) * HALF)
                      if emitM:
                          nc.vector.tensor_scalar(
                              SM[0:72, cols], psM_[h][0:72, :],
                              bM[0:72, 0:1], 0.0, ALU.add, ALU.max,
                          )
                  if emitM and not _NC_CACHE.get("no_dup"):
                      m5 = _NC_CACHE.get("dup5", "dve")
                      if m5 == "sp":
                          nc.sync.dma_start(out=SD[64:96, :], in_=SM[0:32, :])
                      else:
                          nc.vector.tensor_copy(SD[64:96, :], SM[0:32, :])
                  for h in range(NH):
                      c0 = h * HALF
                      if emitD:
                          if h < NH - 1:
                              nc.vector.tensor_scalar(
                                  SD[0:64, c0:c0 + HALF], psD[h][0:64, :],
                                  bD[0:64, 0:1], 0.0, ALU.add, ALU.max,
                              )
                          else:
                              hw_ = HALF // 2
                              nc.vector.tensor_scalar(
                                  SD[0:64, c0:c0 + hw_], psD[h][0:64, 0:hw_],
                                  bD[0:64, 0:1], 0.0, ALU.add, ALU.max,
                              )
                              nc.scalar.activation(
                                  SD[0:64, c0 + hw_:c0 + HALF],
                                  psD[h][0:64, hw_:HALF],
                                  AF.Relu, bias=bD[0:64, 0:1],
                              )
                  for h in range(NH):
                      cols = slice(h * HALF, (h + 1) * HALF)
                      if emitE:
                          nc.scalar.activation(
                              SE[0:79, cols], psE[h][0:79, :], AF.Sigmoid,
                              bias=bE[0:79, 0:1],
                          )

            # final step: t=78 of L7 in batch-partition layout -> out [BL, 79]
            for c in range(16):
                csl = slice(c * 128, (c + 1) * 128)
                psO = pspool.tile([128, 80], fp32, tag="ps", name="psO")
                mm(psO[:, :], SE[0:79, csl], wE[:, :], start=True, stop=False)
                mm(psO[:, :], SD[0:64, csl], wF[:, :], start=False, stop=True)
                ob = opool.tile([128, 80], fp32, tag="ob", name="ob")
                nc.scalar.activation(ob[:, :], psO[:, :], AF.Sigmoid)
                nc.sync.dma_start(out=out_d[csl, :], in_=ob[:, 0:79])

    nc.compile()
    return nc


def _get_nc(reps=1):
    key = ("nc", reps)
    if key not in _NC_CACHE:
        _NC_CACHE[key] = _build_bass(reps)
    return _NC_CACHE[key]


def _pack_inputs(inputs):
    g = lambda k: np.ascontiguousarray(np.asarray(inputs[k], dtype=np.float32))
    Wx = [g(f"Wx{i}") for i in range(8)]
    Wh = [g(f"Wh{i}") for i in range(8)]
    b = [g(f"b{i}") for i in range(8)]

    # pack A [128, 96] x 32 variants; SA rows h1@0:32 h0@32:96 x@96+i;
    # psA cols h1'@0:32 h0'@32:96
    wA = np.zeros((128, NSLOT * 96), np.float32)
    for i in range(NSLOT):
        blk = wA[:, i * 96:(i + 1) * 96]
        blk[0:32, 0:32] = Wh[1]
        blk[32:96, 0:32] = Wx[1]
        blk[32:96, 32:96] = Wh[0]
        blk[96 + i, 32:96] = Wx[0][0]

    # pack M: SM rows h5@0:32 h4@32:48 h3@48:56 h2@56:72 pad@72:96
    # h1dup@96:128; psM cols h5'@0:32 h4'@32:48 h3'@48:56 h2'@56:72
    wM = np.zeros((128, 72), np.float32)
    wM[0:32, 0:32] = Wh[5]
    wM[32:48, 0:32] = Wx[5]
    wM[32:48, 32:48] = Wh[4]
    wM[48:56, 32:48] = Wx[4]
    wM[48:56, 48:56] = Wh[3]
    wM[56:72, 48:56] = Wx[3]
    wM[56:72, 56:72] = Wh[2]
    wM[96:128, 56:72] = Wx[2]

    wD = np.zeros((96, 64), np.float32)
    wD[0:64, :] = Wh[6]
    wD[64:96, :] = Wx[6]

    wE = np.zeros((79, 80), np.float32)
    wE[:, 0:79] = Wh[7]
    wF = np.zeros((64, 80), np.float32)
    wF[:, 0:79] = Wx[7]

    bA = np.concatenate([b[1], b[0]]).reshape(96, 1).astype(np.float32)
    bM = np.concatenate([b[5], b[4], b[3], b[2]]).reshape(72, 1).astype(np.float32)
    bD = b[6].reshape(64, 1).astype(np.float32)
    bE = b[7].reshape(79, 1).astype(np.float32)

    cdt = np.float32
    if _NC_CACHE.get("cdt") != "f32r":
        import ml_dtypes
        cdt = ml_dtypes.bfloat16
        wA, wM, wD, wE, wF = (w.astype(cdt) for w in (wA, wM, wD, wE, wF))

    zz = np.zeros((128, BL), cdt)
    common = dict(wA=wA, wM=wM, wD=wD, wE=wE, wF=wF, bA=bA, bM=bM, bD=bD,
                  bE=bE, zz=zz)

    x = np.asarray(inputs["x"], dtype=np.float32)
    in_maps = []
    for c in range(NCORES):
        xs = x[c * BL:(c + 1) * BL]
        m = dict(common)
        m["xt"] = np.ascontiguousarray(xs.T).astype(cdt)
        in_maps.append(m)
    return in_maps


def run(inputs, trace=False, **kw):
    from concourse.bass_utils import run_bass_kernel_spmd

    nc = _get_nc()
    in_maps = _pack_inputs(inputs)
    res = run_bass_kernel_spmd(nc, in_maps, core_ids=list(range(NCORES)),
                               trace=trace, **kw)
    out = np.concatenate([res.results[c]["out"] for c in range(NCORES)], axis=0)
    return out.astype(np.float32), res


def kernel(**inputs) -> np.ndarray:
    out, _ = run(inputs, trace=False)
    return out
